# revision 24
# baseline (speedup 1.0000x reference)
"""Self-contained Trainium2 kernel for nn_Attention_867583394375.

Data-parallel over batch B=8 across 8 NeuronCores (1 sample/core).
Per core: single-head attention (N=4096 tokens, d=256) + output proj +
global BatchNorm (batch stats all-reduced across cores) + LeakyReLU.
Returns (act, attn) like the reference.
"""

import sys
import types

sys.path.insert(0, "/opt/trn_rl_repo")

import numpy as np

import concourse.bacc as bacc
import concourse.bass as bass
import concourse.tile as tile
from concourse import mybir
from concourse.bass_utils import run_bass_kernel_spmd
from concourse.masks import make_identity

F32 = mybir.dt.float32
BF16 = mybir.dt.bfloat16

N_CORES = 8
B = 8
C = 256            # channels == out_c
NPIX = 4096        # H*W
E = 256            # inner dim
SCALE = E ** -0.5  # 1/16
EPS = 1e-5
NEG_SLOPE = 0.01
JT = 32            # j-tiles of 128


def build(nb=32):
    """Build the SPMD bass graph. nb = number of 128-row i-blocks (32 = full)."""
    nc = bacc.Bacc("TRN2", target_bir_lowering=False, debug=False,
                   num_devices=N_CORES)

    x_d = nc.declare_dram_parameter("x", [C, NPIX], F32, isOutput=False)
    wqk_d = nc.declare_dram_parameter("Wqk", [2 * E, C], F32, isOutput=False)
    wout_d = nc.declare_dram_parameter("Wout", [C, C], F32, isOutput=False)
    bout_d = nc.declare_dram_parameter("bout", [C, 1], F32, isOutput=False)
    gamma_d = nc.declare_dram_parameter("gamma", [C, 1], F32, isOutput=False)
    beta_d = nc.declare_dram_parameter("beta", [C, 1], F32, isOutput=False)
    attn_d = nc.declare_dram_parameter("attn", [NPIX, NPIX], F32, isOutput=True)
    act_d = nc.declare_dram_parameter("act", [C, NPIX], F32, isOutput=True)

    inv_bn = 1.0 / (B * NPIX)

    with tile.TileContext(nc) as tc:
        with (
            tc.tile_pool(name="const", bufs=1) as const,
            tc.tile_pool(name="pbuf", bufs=2) as p_pool,
            tc.tile_pool(name="ptbuf", bufs=2) as pt_pool,
            tc.tile_pool(name="astage", bufs=4) as a_pool,
            tc.tile_pool(name="small", bufs=2) as small,
            tc.tile_pool(name="ps_s", bufs=2, space="PSUM") as ps_s,
            tc.tile_pool(name="ps_t", bufs=3, space="PSUM") as ps_t,
            tc.tile_pool(name="ps_acc", bufs=1, space="PSUM") as ps_acc,
            tc.tile_pool(name="dram", bufs=1, space="DRAM") as dram,
        ):
            # ---------------- setup: loads ----------------
            xs = [const.tile([128, NPIX], F32, tag=f"xs{ch}", name=f"xs{ch}")
                  for ch in range(2)]

            wqk_nat = [const.tile([128, C], F32, tag=f"wqkn{eb}", name=f"wqkn{eb}")
                       for eb in range(4)]
            for eb in range(4):
                nc.sync.dma_start(out=wqk_nat[eb],
                                  in_=wqk_d[eb * 128:(eb + 1) * 128, :])
            wout_nat = [const.tile([128, C], F32, tag=f"woutn{ob}", name=f"woutn{ob}")
                        for ob in range(2)]
            for ob in range(2):
                nc.sync.dma_start(out=wout_nat[ob],
                                  in_=wout_d[ob * 128:(ob + 1) * 128, :])

            bout_sb = [const.tile([128, 1], F32, tag=f"bout{ob}", name=f"bout{ob}") for ob in range(2)]
            gamma_sb = [const.tile([128, 1], F32, tag=f"gamma{ob}", name=f"gamma{ob}") for ob in range(2)]
            beta_sb = [const.tile([128, 1], F32, tag=f"beta{ob}", name=f"beta{ob}") for ob in range(2)]
            for ob in range(2):
                sl = slice(ob * 128, (ob + 1) * 128)
                nc.sync.dma_start(out=bout_sb[ob], in_=bout_d[sl, :])
                nc.sync.dma_start(out=gamma_sb[ob], in_=gamma_d[sl, :])
                nc.sync.dma_start(out=beta_sb[ob], in_=beta_d[sl, :])

            ident = const.tile([128, 128], F32, tag="ident")
            make_identity(nc, ident)
            ident_b = const.tile([128, 128], BF16, tag="identb")
            nc.scalar.copy(ident_b, ident)

            # ---------------- setup: transposes (f32, PE) ----------------
            # WqkT[cb]: [c(128), e(512)] bf16 ; WoutT[cb]: [c(128), o(256)] bf16
            wqkT = [const.tile([128, 512], BF16, tag=f"wqkT{cb}", name=f"wqkT{cb}") for cb in range(2)]
            woutT = [const.tile([128, C], BF16, tag=f"woutT{cb}", name=f"woutT{cb}") for cb in range(2)]
            for eb in range(4):
                for cb in range(2):
                    pst = ps_t.tile([128, 128], F32, tag="pst", name="pstf")
                    nc.tensor.transpose(pst, wqk_nat[eb][:, cb * 128:(cb + 1) * 128],
                                        ident)
                    nc.scalar.copy(wqkT[cb][:, eb * 128:(eb + 1) * 128], pst)
            for ob in range(2):
                for cb in range(2):
                    pst = ps_t.tile([128, 128], F32, tag="pst", name="pstf")
                    nc.tensor.transpose(pst, wout_nat[ob][:, cb * 128:(cb + 1) * 128],
                                        ident)
                    nc.scalar.copy(woutT[cb][:, ob * 128:(ob + 1) * 128], pst)

            # pipelined: per 512-col chunk, DMA x -> cast bf16 -> 4 projections
            xs_b = [const.tile([128, NPIX], BF16, tag=f"xsb{ch}", name=f"xsb{ch}")
                    for ch in range(2)]
            qkT = [const.tile([128, NPIX], BF16, tag=f"qkT{eb}", name=f"qkT{eb}")
                   for eb in range(4)]
            for ncg in range(NPIX // 512):
                sl = slice(ncg * 512, (ncg + 1) * 512)
                for ch in range(2):
                    nc.sync.dma_start(out=xs[ch][:, sl],
                                      in_=x_d[ch * 128:(ch + 1) * 128, sl])
                    if ch == 0:
                        nc.scalar.copy(xs_b[ch][:, sl], xs[ch][:, sl])
                    else:
                        nc.vector.tensor_copy(xs_b[ch][:, sl], xs[ch][:, sl])
                for eb in range(4):
                    ps = ps_s.tile([128, 512], F32, tag="ps", name="ps_proj")
                    for cb in range(2):
                        nc.tensor.matmul(
                            ps,
                            wqkT[cb][:, eb * 128:(eb + 1) * 128],
                            xs_b[cb][:, sl],
                            start=(cb == 0), stop=(cb == 1),
                        )
                    if eb % 2 == 0:
                        nc.scalar.copy(qkT[eb][:, sl], ps)
                    else:
                        nc.vector.tensor_copy(qkT[eb][:, sl], ps)

            # VW[jt]: [j(128), o(256)] bf16 = (nf @ Wout^T) tiles
            # y = P @ VW / rowsum + bout  ==  (attn @ nf) @ Wout^T + bout
            VW = const.tile([128, JT * C], BF16, tag="VW", name="VW")
            for jt in range(JT):
                ps = ps_s.tile([128, 1024], F32, tag="ps", name="ps_vw")
                for cb in range(2):
                    nc.tensor.matmul(
                        ps[:, :C],
                        xs_b[cb][:, jt * 128:(jt + 1) * 128],
                        woutT[cb],
                        start=(cb == 0), stop=(cb == 1),
                    )
                if jt % 2 == 0:
                    nc.scalar.copy(VW[:, jt * C:(jt + 1) * C], ps[:, :C])
                else:
                    nc.vector.tensor_copy(VW[:, jt * C:(jt + 1) * C], ps[:, :C])

            # y (pre-BN) [o, n] f32, and per-block stats
            y_sb = [const.tile([128, NPIX], F32, tag=f"y{ob}", name=f"y{ob}") for ob in range(2)]
            ysum = [const.tile([128, 32], F32, tag=f"ysum{ob}", name=f"ysum{ob}") for ob in range(2)]
            ysq = [const.tile([128, 32], F32, tag=f"ysq{ob}", name=f"ysq{ob}") for ob in range(2)]

            # ---------------- main loop (1-block software skew, interleaved) ----------------
            pending = []
            for ib in range(nb + 1):
                cur = None
                if ib < nb:
                    cur = (ib,
                           p_pool.tile([128, NPIX], BF16, tag="pbuf", name="P"),
                           small.tile([128, 4], F32, tag="rs", name="rs"))
                prev = pending.pop(0) if pending else None

                recip = None
                out_ps = None
                PT = None
                if prev is not None:
                    pib, Pp, rsp = prev
                    rowsum = small.tile([128, 1], F32, tag="rowsum", name="rowsum")
                    nc.vector.reduce_sum(out=rowsum, in_=rsp,
                                         axis=mybir.AxisListType.X)
                    recip = small.tile([128, 1], F32, tag="recip", name="recip")
                    nc.vector.reciprocal(out=recip, in_=rowsum)
                    PT = pt_pool.tile([128, NPIX], BF16, tag="ptbuf", name="PT")
                    out_ps = ps_acc.tile([128, C], F32, tag="psacc", name="out_ps")

                for jc2 in range(4):
                    if cur is not None:
                        cib, P, rs = cur
                        ps = ps_s.tile([128, 1024], F32, tag="ps", name="ps_qk")
                        for eb in range(2):
                            for half in range(2):
                                j0 = jc2 * 1024 + half * 512
                                nc.tensor.matmul(
                                    ps[:, half * 512:(half + 1) * 512],
                                    qkT[eb][:, cib * 128:(cib + 1) * 128],
                                    qkT[2 + eb][:, j0:j0 + 512],
                                    start=(eb == 0), stop=(eb == 1),
                                )
                        nc.scalar.activation(
                            P[:, jc2 * 1024:(jc2 + 1) * 1024], ps,
                            mybir.ActivationFunctionType.Exp,
                            scale=SCALE,
                            accum_out=rs[:, jc2:jc2 + 1],
                        )
                    if prev is not None:
                        pib, Pp, rsp = prev
                        # two transpose groups of 4 + PSUM->SBUF copy
                        for g in range(2):
                            jq = jc2 * 2 + g
                            pst = ps_t.tile([128, 512], BF16, tag="pst", name="pst")
                            for t in range(4):
                                jt = jq * 4 + t
                                nc.tensor.transpose(
                                    pst[:, t * 128:(t + 1) * 128],
                                    Pp[:, jt * 128:(jt + 1) * 128],
                                    ident_b,
                                )
                            if jq % 2 == 0:
                                nc.scalar.copy(PT[:, jq * 512:(jq + 1) * 512], pst)
                            else:
                                nc.vector.tensor_copy(PT[:, jq * 512:(jq + 1) * 512], pst)
                        # 8 PV accumulation matmuls (directly into y[i, o])
                        for jt in range(jc2 * 8, jc2 * 8 + 8):
                            nc.tensor.matmul(
                                out_ps,
                                PT[:, jt * 128:(jt + 1) * 128],
                                VW[:, jt * C:(jt + 1) * C],
                                start=(jt == 0), stop=(jt == JT - 1),
                            )
                        # normalized f32 attn chunk -> DRAM
                        ast = a_pool.tile([128, 1024], F32, tag="astage", name="astage")
                        nc.vector.tensor_scalar_mul(
                            ast, Pp[:, jc2 * 1024:(jc2 + 1) * 1024], recip)
                        nc.sync.dma_start(
                            out=attn_d[pib * 128:(pib + 1) * 128,
                                       jc2 * 1024:(jc2 + 1) * 1024],
                            in_=ast)

                if prev is not None:
                    pib, Pp, rsp = prev
                    # y[i, o]/rowsum, cast bf16
                    out_sb = small.tile([128, C], BF16, tag="outsb", name="out_sb")
                    nc.vector.tensor_scalar_mul(out_sb, out_ps, recip)

                    # transpose y -> [o, i]; then bias + stats
                    for ob in range(2):
                        pst = ps_t.tile([128, 128], BF16, tag="pst", name="pst")
                        nc.tensor.transpose(
                            pst, out_sb[:, ob * 128:(ob + 1) * 128], ident_b)
                        ysl = y_sb[ob][:, pib * 128:(pib + 1) * 128]
                        nc.vector.tensor_scalar(
                            out=ysl, in0=pst,
                            scalar1=bout_sb[ob], scalar2=0.0,
                            op0=mybir.AluOpType.add,
                            op1=mybir.AluOpType.add,
                            accum_out=ysum[ob][:, pib:pib + 1],
                        )
                        sq = small.tile([128, 128], F32, tag="sq", name="sq")
                        nc.scalar.activation(
                            sq, ysl,
                            mybir.ActivationFunctionType.Square,
                            accum_out=ysq[ob][:, pib:pib + 1],
                        )

                if cur is not None:
                    pending.append(cur)

            # ---------------- BN stats + AllReduce (gpsimd queues) ----------------
            bn_in = dram.tile([128, 4], F32, tag="bn_in", name="bn_in")
            bn_out = dram.tile([128, 4], F32, tag="bn_out", name="bn_out")
            stats_loc = small.tile([128, 4], F32, tag="statsl", name="stats_loc")
            for ob in range(2):
                nc.vector.reduce_sum(out=stats_loc[:, ob:ob + 1], in_=ysum[ob][:, :nb],
                                     axis=mybir.AxisListType.X)
                nc.vector.reduce_sum(out=stats_loc[:, 2 + ob:3 + ob], in_=ysq[ob][:, :nb],
                                     axis=mybir.AxisListType.X)
            nc.gpsimd.dma_start(out=bn_in, in_=stats_loc)
            nc.gpsimd.collective_compute(
                "AllReduce",
                mybir.AluOpType.add,
                replica_groups=[list(range(N_CORES))],
                ins=[bn_in.opt()],
                outs=[bn_out.opt()],
            )
            stats_sb = small.tile([128, 4], F32, tag="statsg", name="stats_sb")
            nc.gpsimd.dma_start(out=stats_sb, in_=bn_out)

            # ---------------- BN epilogue + LeakyReLU ----------------
            for ob in range(2):
                mean = small.tile([128, 1], F32, tag="mean", name="mean")
                ey2 = small.tile([128, 1], F32, tag="ey2", name="ey2")
                nc.vector.tensor_scalar_mul(mean, stats_sb[:, ob:ob + 1], inv_bn)
                nc.vector.tensor_scalar_mul(ey2, stats_sb[:, 2 + ob:3 + ob], inv_bn)
                var = small.tile([128, 1], F32, tag="var", name="var")
                nc.vector.tensor_tensor(out=var, in0=mean, in1=mean,
                                        op=mybir.AluOpType.mult)
                nc.vector.tensor_tensor(out=var, in0=ey2, in1=var,
                                        op=mybir.AluOpType.subtract)
                nc.vector.tensor_scalar_add(var, var, EPS)
                sd = small.tile([128, 1], F32, tag="sd", name="sd")
                nc.scalar.sqrt(sd, var)
                rstd = small.tile([128, 1], F32, tag="rstd", name="rstd")
                nc.vector.reciprocal(out=rstd, in_=sd)
                scale_t = small.tile([128, 1], F32, tag="scalet", name="scale_t")
                nc.vector.tensor_tensor(out=scale_t, in0=gamma_sb[ob], in1=rstd,
                                        op=mybir.AluOpType.mult)
                ms = small.tile([128, 1], F32, tag="ms", name="ms")
                nc.vector.tensor_tensor(out=ms, in0=mean, in1=scale_t,
                                        op=mybir.AluOpType.mult)
                shift = small.tile([128, 1], F32, tag="shift", name="shift")
                nc.vector.tensor_tensor(out=shift, in0=beta_sb[ob], in1=ms,
                                        op=mybir.AluOpType.subtract)

                ncols = nb * 128
                yhat = p_pool.tile([128, NPIX], F32, tag="yhat", name="yhat",
                                   bufs=1)
                for c0 in range(0, ncols, 512):
                    w = min(512, ncols - c0)
                    sl = slice(c0, c0 + w)
                    nc.scalar.activation(
                        yhat[:, sl], y_sb[ob][:, sl],
                        mybir.ActivationFunctionType.Lrelu,
                        bias=shift, scale=scale_t, alpha=NEG_SLOPE,
                    )
                    nc.sync.dma_start(out=act_d[ob * 128:(ob + 1) * 128, sl],
                                      in_=yhat[:, sl])

    nc.compile()
    return nc


_CACHE = {}


def _get_nc(nb=32):
    if nb not in _CACHE:
        _CACHE[nb] = build(nb)
    return _CACHE[nb]


def _register_ntff_shim():
    """antenv.axon_hooks is missing from this image; shim it so trace=True works."""
    try:
        import antenv.axon_hooks  # noqa: F401
        return
    except ImportError:
        pass
    import antenv  # noqa: F401
    mod = types.ModuleType("antenv.axon_hooks")
    _hook = [None]
    mod.set_axon_ntff_profile_hook = lambda h: _hook.__setitem__(0, h)
    mod.get_axon_ntff_profile_hook = lambda: _hook[0]
    sys.modules["antenv.axon_hooks"] = mod
    try:
        from trn_agent_boot.trn_boot import _ntff_profile_via_ctypes
        mod.set_axon_ntff_profile_hook(
            _ntff_profile_via_ctypes("/opt/axon/libaxon_pjrt.so"))
    except Exception:
        pass


def run(inputs, trace=False, nb=32):
    nc = _get_nc(nb)
    x = np.ascontiguousarray(np.asarray(inputs["x"], dtype=np.float32))
    wqk = np.ascontiguousarray(np.asarray(inputs["Wqk"], dtype=np.float32))
    wout = np.ascontiguousarray(np.asarray(inputs["Wout"], dtype=np.float32))
    bout = np.asarray(inputs["bout"], dtype=np.float32).reshape(C, 1).copy()
    gamma = np.asarray(inputs["gamma"], dtype=np.float32).reshape(C, 1).copy()
    beta = np.asarray(inputs["beta"], dtype=np.float32).reshape(C, 1).copy()

    in_maps = [
        {
            "x": np.ascontiguousarray(x[b].reshape(C, NPIX)),
            "Wqk": wqk, "Wout": wout,
            "bout": bout, "gamma": gamma, "beta": beta,
        }
        for b in range(B)
    ]
    if trace:
        _register_ntff_shim()
    res = run_bass_kernel_spmd(nc, in_maps, core_ids=list(range(N_CORES)),
                               trace=trace)
    act = np.stack([res.results[b]["act"].reshape(C, 64, 64) for b in range(B)])
    attn = np.stack([res.results[b]["attn"] for b in range(B)])
    return act, attn, res.exec_time_ns


def kernel(**inputs):
    act, attn, _ = run(inputs, trace=False)
    return act, attn


if __name__ == "__main__":
    rng = np.random.default_rng(0)
    ins = {
        "x": rng.standard_normal((B, C, 64, 64), dtype=np.float32),
        "Wqk": (rng.standard_normal((512, C)) * 0.02).astype(np.float32),
        "Wout": (rng.standard_normal((C, C)) * 0.02).astype(np.float32),
        "bout": np.zeros(C, np.float32),
        "gamma": np.ones(C, np.float32),
        "beta": np.zeros(C, np.float32),
    }
    act, attn, t = run(ins, trace=False)
    print("act", act.shape, "attn", attn.shape, "t", t)


# revision 25
# speedup vs baseline: 1.0140x; 1.0140x over previous
"""Self-contained Trainium2 kernel for nn_Attention_867583394375.

Data-parallel over batch B=8 across 8 NeuronCores (1 sample/core).
Per core: single-head attention (N=4096 tokens, d=256) + output proj +
global BatchNorm (batch stats all-reduced across cores) + LeakyReLU.
Returns (act, attn) like the reference.
"""

import sys
import types

sys.path.insert(0, "/opt/trn_rl_repo")

import numpy as np

import concourse.bacc as bacc
import concourse.bass as bass
import concourse.tile as tile
from concourse import mybir
from concourse.bass_utils import run_bass_kernel_spmd
from concourse.masks import make_identity

F32 = mybir.dt.float32
BF16 = mybir.dt.bfloat16

N_CORES = 8
B = 8
C = 256            # channels == out_c
NPIX = 4096        # H*W
E = 256            # inner dim
SCALE = E ** -0.5  # 1/16
EPS = 1e-5
NEG_SLOPE = 0.01
JT = 32            # j-tiles of 128


def build(nb=32):
    """Build the SPMD bass graph. nb = number of 128-row i-blocks (32 = full)."""
    nc = bacc.Bacc("TRN2", target_bir_lowering=False, debug=False,
                   num_devices=N_CORES)

    x_d = nc.declare_dram_parameter("x", [C, NPIX], F32, isOutput=False)
    wqk_d = nc.declare_dram_parameter("Wqk", [2 * E, C], F32, isOutput=False)
    wout_d = nc.declare_dram_parameter("Wout", [C, C], F32, isOutput=False)
    bout_d = nc.declare_dram_parameter("bout", [C, 1], F32, isOutput=False)
    gamma_d = nc.declare_dram_parameter("gamma", [C, 1], F32, isOutput=False)
    beta_d = nc.declare_dram_parameter("beta", [C, 1], F32, isOutput=False)
    attn_d = nc.declare_dram_parameter("attn", [NPIX, NPIX], F32, isOutput=True)
    act_d = nc.declare_dram_parameter("act", [C, NPIX], F32, isOutput=True)

    inv_bn = 1.0 / (B * NPIX)

    with tile.TileContext(nc) as tc:
        with (
            tc.tile_pool(name="const", bufs=1) as const,
            tc.tile_pool(name="pbuf", bufs=2) as p_pool,
            tc.tile_pool(name="ptbuf", bufs=2) as pt_pool,
            tc.tile_pool(name="astage", bufs=4) as a_pool,
            tc.tile_pool(name="small", bufs=2) as small,
            tc.tile_pool(name="ps_s", bufs=2, space="PSUM") as ps_s,
            tc.tile_pool(name="ps_t", bufs=3, space="PSUM") as ps_t,
            tc.tile_pool(name="ps_acc", bufs=1, space="PSUM") as ps_acc,
            tc.tile_pool(name="dram", bufs=1, space="DRAM") as dram,
        ):
            # ---------------- setup: loads ----------------
            xs = [const.tile([128, NPIX], F32, tag=f"xs{ch}", name=f"xs{ch}")
                  for ch in range(2)]

            wqk_nat = [const.tile([128, C], F32, tag=f"wqkn{eb}", name=f"wqkn{eb}")
                       for eb in range(4)]
            for eb in range(4):
                nc.sync.dma_start(out=wqk_nat[eb],
                                  in_=wqk_d[eb * 128:(eb + 1) * 128, :])
            wout_nat = [const.tile([128, C], F32, tag=f"woutn{ob}", name=f"woutn{ob}")
                        for ob in range(2)]
            for ob in range(2):
                nc.sync.dma_start(out=wout_nat[ob],
                                  in_=wout_d[ob * 128:(ob + 1) * 128, :])

            bout_sb = [const.tile([128, 1], F32, tag=f"bout{ob}", name=f"bout{ob}") for ob in range(2)]
            gamma_sb = [const.tile([128, 1], F32, tag=f"gamma{ob}", name=f"gamma{ob}") for ob in range(2)]
            beta_sb = [const.tile([128, 1], F32, tag=f"beta{ob}", name=f"beta{ob}") for ob in range(2)]
            for ob in range(2):
                sl = slice(ob * 128, (ob + 1) * 128)
                nc.sync.dma_start(out=bout_sb[ob], in_=bout_d[sl, :])
                nc.sync.dma_start(out=gamma_sb[ob], in_=gamma_d[sl, :])
                nc.sync.dma_start(out=beta_sb[ob], in_=beta_d[sl, :])

            ident = const.tile([128, 128], F32, tag="ident")
            make_identity(nc, ident)
            ident_b = const.tile([128, 128], BF16, tag="identb")
            nc.scalar.copy(ident_b, ident)

            # ---------------- setup: transposes (f32, PE) ----------------
            # WqkT[cb]: [c(128), e(512)] bf16 ; WoutT[cb]: [c(128), o(256)] bf16
            wqkT = [const.tile([128, 512], BF16, tag=f"wqkT{cb}", name=f"wqkT{cb}") for cb in range(2)]
            woutT = [const.tile([128, C], BF16, tag=f"woutT{cb}", name=f"woutT{cb}") for cb in range(2)]
            for eb in range(4):
                for cb in range(2):
                    pst = ps_t.tile([128, 128], F32, tag="pst", name="pstf")
                    nc.tensor.transpose(pst, wqk_nat[eb][:, cb * 128:(cb + 1) * 128],
                                        ident)
                    nc.scalar.copy(wqkT[cb][:, eb * 128:(eb + 1) * 128], pst)
            for ob in range(2):
                for cb in range(2):
                    pst = ps_t.tile([128, 128], F32, tag="pst", name="pstf")
                    nc.tensor.transpose(pst, wout_nat[ob][:, cb * 128:(cb + 1) * 128],
                                        ident)
                    nc.scalar.copy(woutT[cb][:, ob * 128:(ob + 1) * 128], pst)

            # pipelined: per 512-col chunk, DMA x -> cast bf16 -> 4 projections
            xs_b = [const.tile([128, NPIX], BF16, tag=f"xsb{ch}", name=f"xsb{ch}")
                    for ch in range(2)]
            qkT = [const.tile([128, NPIX], BF16, tag=f"qkT{eb}", name=f"qkT{eb}")
                   for eb in range(4)]
            for ncg in range(NPIX // 512):
                sl = slice(ncg * 512, (ncg + 1) * 512)
                for ch in range(2):
                    nc.sync.dma_start(out=xs[ch][:, sl],
                                      in_=x_d[ch * 128:(ch + 1) * 128, sl])
                    if ch == 0:
                        nc.scalar.copy(xs_b[ch][:, sl], xs[ch][:, sl])
                    else:
                        nc.vector.tensor_copy(xs_b[ch][:, sl], xs[ch][:, sl])
                for eb in range(4):
                    ps = ps_s.tile([128, 512], F32, tag="ps", name="ps_proj")
                    for cb in range(2):
                        nc.tensor.matmul(
                            ps,
                            wqkT[cb][:, eb * 128:(eb + 1) * 128],
                            xs_b[cb][:, sl],
                            start=(cb == 0), stop=(cb == 1),
                        )
                    if eb % 2 == 0:
                        nc.scalar.copy(qkT[eb][:, sl], ps)
                    else:
                        nc.vector.tensor_copy(qkT[eb][:, sl], ps)

            # VW[jt]: [j(128), o(256)] bf16 = (nf @ Wout^T) tiles
            # y = P @ VW / rowsum + bout  ==  (attn @ nf) @ Wout^T + bout
            VW = const.tile([128, JT * C], BF16, tag="VW", name="VW")
            for jt in range(JT):
                ps = ps_s.tile([128, 1024], F32, tag="ps", name="ps_vw")
                for cb in range(2):
                    nc.tensor.matmul(
                        ps[:, :C],
                        xs_b[cb][:, jt * 128:(jt + 1) * 128],
                        woutT[cb],
                        start=(cb == 0), stop=(cb == 1),
                    )
                if jt % 2 == 0:
                    nc.scalar.copy(VW[:, jt * C:(jt + 1) * C], ps[:, :C])
                else:
                    nc.vector.tensor_copy(VW[:, jt * C:(jt + 1) * C], ps[:, :C])

            # y (pre-BN) [o, n] f32, and per-block stats
            y_sb = [const.tile([128, NPIX], F32, tag=f"y{ob}", name=f"y{ob}") for ob in range(2)]
            ysum = [const.tile([128, 32], F32, tag=f"ysum{ob}", name=f"ysum{ob}") for ob in range(2)]
            ysq = [const.tile([128, 32], F32, tag=f"ysq{ob}", name=f"ysq{ob}") for ob in range(2)]

            # ---------------- main loop (1-block software skew, interleaved) ----------------
            pending = []
            for ib in range(nb + 1):
                cur = None
                if ib < nb:
                    cur = (ib,
                           p_pool.tile([128, NPIX], BF16, tag="pbuf", name="P"),
                           small.tile([128, 4], F32, tag="rs", name="rs"))
                prev = pending.pop(0) if pending else None

                recip = None
                out_ps = None
                PT = None
                if prev is not None:
                    pib, Pp, rsp = prev
                    rowsum = small.tile([128, 1], F32, tag="rowsum", name="rowsum")
                    nc.vector.reduce_sum(out=rowsum, in_=rsp,
                                         axis=mybir.AxisListType.X)
                    recip = small.tile([128, 1], F32, tag="recip", name="recip")
                    nc.vector.reciprocal(out=recip, in_=rowsum)
                    PT = pt_pool.tile([128, NPIX], BF16, tag="ptbuf", name="PT")
                    out_ps = ps_acc.tile([128, C], F32, tag="psacc", name="out_ps")

                for jc2 in range(4):
                    if cur is not None:
                        cib, P, rs = cur
                        ps = ps_s.tile([128, 1024], F32, tag="ps", name="ps_qk")
                        for eb in range(2):
                            for half in range(2):
                                j0 = jc2 * 1024 + half * 512
                                nc.tensor.matmul(
                                    ps[:, half * 512:(half + 1) * 512],
                                    qkT[eb][:, cib * 128:(cib + 1) * 128],
                                    qkT[2 + eb][:, j0:j0 + 512],
                                    start=(eb == 0), stop=(eb == 1),
                                )
                        nc.scalar.activation(
                            P[:, jc2 * 1024:(jc2 + 1) * 1024], ps,
                            mybir.ActivationFunctionType.Exp,
                            scale=SCALE,
                            accum_out=rs[:, jc2:jc2 + 1],
                        )
                    if prev is not None:
                        pib, Pp, rsp = prev
                        # two transpose groups of 4 + PSUM->SBUF copy
                        for g in range(2):
                            jq = jc2 * 2 + g
                            pst = ps_t.tile([128, 512], BF16, tag="pst", name="pst")
                            for t in range(4):
                                jt = jq * 4 + t
                                nc.tensor.transpose(
                                    pst[:, t * 128:(t + 1) * 128],
                                    Pp[:, jt * 128:(jt + 1) * 128],
                                    ident_b,
                                )
                            if jq % 2 == 0:
                                nc.scalar.copy(PT[:, jq * 512:(jq + 1) * 512], pst)
                            else:
                                nc.vector.tensor_copy(PT[:, jq * 512:(jq + 1) * 512], pst)
                        # 8 PV accumulation matmuls (directly into y[i, o])
                        for jt in range(jc2 * 8, jc2 * 8 + 8):
                            nc.tensor.matmul(
                                out_ps,
                                PT[:, jt * 128:(jt + 1) * 128],
                                VW[:, jt * C:(jt + 1) * C],
                                start=(jt == 0), stop=(jt == JT - 1),
                            )
                        # normalized f32 attn chunk -> DRAM
                        ast = a_pool.tile([128, 1024], F32, tag="astage", name="astage")
                        nc.vector.tensor_scalar_mul(
                            ast, Pp[:, jc2 * 1024:(jc2 + 1) * 1024], recip)
                        nc.sync.dma_start(
                            out=attn_d[pib * 128:(pib + 1) * 128,
                                       jc2 * 1024:(jc2 + 1) * 1024],
                            in_=ast)

                if prev is not None:
                    pib, Pp, rsp = prev
                    # y[i, o]/rowsum, cast bf16
                    out_sb = small.tile([128, C], BF16, tag="outsb", name="out_sb")
                    nc.vector.tensor_scalar_mul(out_sb, out_ps, recip)

                    # transpose y -> [o, i]; then bias + stats
                    for ob in range(2):
                        pst = ps_t.tile([128, 128], BF16, tag="pst", name="pst")
                        nc.tensor.transpose(
                            pst, out_sb[:, ob * 128:(ob + 1) * 128], ident_b)
                        ysl = y_sb[ob][:, pib * 128:(pib + 1) * 128]
                        nc.vector.tensor_scalar(
                            out=ysl, in0=pst,
                            scalar1=bout_sb[ob], scalar2=0.0,
                            op0=mybir.AluOpType.add,
                            op1=mybir.AluOpType.add,
                            accum_out=ysum[ob][:, pib:pib + 1],
                        )
                        sq = small.tile([128, 128], F32, tag="sq", name="sq")
                        nc.scalar.activation(
                            sq, ysl,
                            mybir.ActivationFunctionType.Square,
                            accum_out=ysq[ob][:, pib:pib + 1],
                        )

                if cur is not None:
                    pending.append(cur)

            # ---------------- BN stats + AllReduce (gpsimd queues) ----------------
            bn_in = dram.tile([128, 4], F32, tag="bn_in", name="bn_in")
            bn_out = dram.tile([128, 4], F32, tag="bn_out", name="bn_out")
            stats_loc = small.tile([128, 4], F32, tag="statsl", name="stats_loc")
            for ob in range(2):
                nc.vector.reduce_sum(out=stats_loc[:, ob:ob + 1], in_=ysum[ob][:, :nb],
                                     axis=mybir.AxisListType.X)
                nc.vector.reduce_sum(out=stats_loc[:, 2 + ob:3 + ob], in_=ysq[ob][:, :nb],
                                     axis=mybir.AxisListType.X)
            nc.gpsimd.dma_start(out=bn_in, in_=stats_loc)
            nc.gpsimd.collective_compute(
                "AllReduce",
                mybir.AluOpType.add,
                replica_groups=[list(range(N_CORES))],
                ins=[bn_in.opt()],
                outs=[bn_out.opt()],
            )
            stats_sb = small.tile([128, 4], F32, tag="statsg", name="stats_sb")
            nc.gpsimd.dma_start(out=stats_sb, in_=bn_out)

            # ---------------- BN epilogue + LeakyReLU ----------------
            for ob in range(2):
                mean = small.tile([128, 1], F32, tag="mean", name="mean")
                ey2 = small.tile([128, 1], F32, tag="ey2", name="ey2")
                nc.vector.tensor_scalar_mul(mean, stats_sb[:, ob:ob + 1], inv_bn)
                nc.vector.tensor_scalar_mul(ey2, stats_sb[:, 2 + ob:3 + ob], inv_bn)
                var = small.tile([128, 1], F32, tag="var", name="var")
                nc.vector.tensor_tensor(out=var, in0=mean, in1=mean,
                                        op=mybir.AluOpType.mult)
                nc.vector.tensor_tensor(out=var, in0=ey2, in1=var,
                                        op=mybir.AluOpType.subtract)
                nc.vector.tensor_scalar_add(var, var, EPS)
                sd = small.tile([128, 1], F32, tag="sd", name="sd")
                nc.scalar.sqrt(sd, var)
                rstd = small.tile([128, 1], F32, tag="rstd", name="rstd")
                nc.vector.reciprocal(out=rstd, in_=sd)
                scale_t = small.tile([128, 1], F32, tag="scalet", name="scale_t")
                nc.vector.tensor_tensor(out=scale_t, in0=gamma_sb[ob], in1=rstd,
                                        op=mybir.AluOpType.mult)
                ms = small.tile([128, 1], F32, tag="ms", name="ms")
                nc.vector.tensor_tensor(out=ms, in0=mean, in1=scale_t,
                                        op=mybir.AluOpType.mult)
                shift = small.tile([128, 1], F32, tag="shift", name="shift")
                nc.vector.tensor_tensor(out=shift, in0=beta_sb[ob], in1=ms,
                                        op=mybir.AluOpType.subtract)

                ncols = nb * 128
                yhat = p_pool.tile([128, NPIX], F32, tag="yhat", name="yhat",
                                   bufs=1)
                for c0 in range(0, ncols, 2048):
                    w = min(2048, ncols - c0)
                    sl = slice(c0, c0 + w)
                    nc.scalar.activation(
                        yhat[:, sl], y_sb[ob][:, sl],
                        mybir.ActivationFunctionType.Lrelu,
                        bias=shift, scale=scale_t, alpha=NEG_SLOPE,
                    )
                    nc.sync.dma_start(out=act_d[ob * 128:(ob + 1) * 128, sl],
                                      in_=yhat[:, sl])

    nc.compile()
    return nc


_CACHE = {}


def _get_nc(nb=32):
    if nb not in _CACHE:
        _CACHE[nb] = build(nb)
    return _CACHE[nb]


def _register_ntff_shim():
    """antenv.axon_hooks is missing from this image; shim it so trace=True works."""
    try:
        import antenv.axon_hooks  # noqa: F401
        return
    except ImportError:
        pass
    import antenv  # noqa: F401
    mod = types.ModuleType("antenv.axon_hooks")
    _hook = [None]
    mod.set_axon_ntff_profile_hook = lambda h: _hook.__setitem__(0, h)
    mod.get_axon_ntff_profile_hook = lambda: _hook[0]
    sys.modules["antenv.axon_hooks"] = mod
    try:
        from trn_agent_boot.trn_boot import _ntff_profile_via_ctypes
        mod.set_axon_ntff_profile_hook(
            _ntff_profile_via_ctypes("/opt/axon/libaxon_pjrt.so"))
    except Exception:
        pass


def run(inputs, trace=False, nb=32):
    nc = _get_nc(nb)
    x = np.ascontiguousarray(np.asarray(inputs["x"], dtype=np.float32))
    wqk = np.ascontiguousarray(np.asarray(inputs["Wqk"], dtype=np.float32))
    wout = np.ascontiguousarray(np.asarray(inputs["Wout"], dtype=np.float32))
    bout = np.asarray(inputs["bout"], dtype=np.float32).reshape(C, 1).copy()
    gamma = np.asarray(inputs["gamma"], dtype=np.float32).reshape(C, 1).copy()
    beta = np.asarray(inputs["beta"], dtype=np.float32).reshape(C, 1).copy()

    in_maps = [
        {
            "x": np.ascontiguousarray(x[b].reshape(C, NPIX)),
            "Wqk": wqk, "Wout": wout,
            "bout": bout, "gamma": gamma, "beta": beta,
        }
        for b in range(B)
    ]
    if trace:
        _register_ntff_shim()
    res = run_bass_kernel_spmd(nc, in_maps, core_ids=list(range(N_CORES)),
                               trace=trace)
    act = np.stack([res.results[b]["act"].reshape(C, 64, 64) for b in range(B)])
    attn = np.stack([res.results[b]["attn"] for b in range(B)])
    return act, attn, res.exec_time_ns


def kernel(**inputs):
    act, attn, _ = run(inputs, trace=False)
    return act, attn


if __name__ == "__main__":
    rng = np.random.default_rng(0)
    ins = {
        "x": rng.standard_normal((B, C, 64, 64), dtype=np.float32),
        "Wqk": (rng.standard_normal((512, C)) * 0.02).astype(np.float32),
        "Wout": (rng.standard_normal((C, C)) * 0.02).astype(np.float32),
        "bout": np.zeros(C, np.float32),
        "gamma": np.ones(C, np.float32),
        "beta": np.zeros(C, np.float32),
    }
    act, attn, t = run(ins, trace=False)
    print("act", act.shape, "attn", attn.shape, "t", t)
